# revision 1
# baseline (speedup 1.0000x reference)
"""Trainium2 Bass kernel: batched 64-digit base-10 addition (nn_Adder).

The reference RNN scan is just carry-propagating decimal addition:
    s_e = a_e + b_e; v_e = s_e + c_e; c_{e+1} = [v_e >= 10];
    digit_e = v_e mod 10   (digits stored MSB-first, carries run LSB->MSB)

Mapping onto one NeuronCore (pure data parallel across 8 cores, batch
524288 -> 65536 rows/core):

  * The inputs are base-10 digits (0..9). Stored as f32 they are 4x
    excess HBM traffic, and measured across several engine mixes the
    f32 kernel is pinned at the sustained HBM wall (~311 GB/s/core,
    ~161us for 50.3 MB/core). The kernel therefore ships the inputs to
    the device as float8e4 (e4m3; integers <= 16 are exact, so the cast
    at the kernel boundary is lossless) and reads 4.19 MB per input per
    core. All arithmetic still happens on device; the f32 OUTPUT tensor
    is produced on device and DMA'd out in full (16.78 MB/core).
  * G=32 rows are packed per SBUF partition along the free dim, with a
    zero separator column before each 64-digit group. At a separator the
    scan state is the previous group's carry (0 or 1) < 10, so the carry
    into the next group's LSB is 0 -> ONE tensor_tensor_scan instruction
    carries 128*G rows.
  * s = a + b runs on the TensorEngine as two accumulating fp8 identity
    matmuls into PSUM (psum = I@a, psum += I@b), 512-column chunks (one
    PSUM bank each). ACT drains PSUM into the LSB-first separator
    layout as bf16 (the MSB<->LSB reversal is folded into its access
    pattern). The first two (small) tiles instead compute s with a DVE
    tensor_tensor add so the first scan is not gated on the
    PE->PSUM->drain chain during pipeline fill.
  * DVE runs the carry chain
        v_t = [10 <= v_{t-1}] + s_t      (op0=is_le, op1=add)
    with bf16 output (values <= 19, exact), then digit extraction in
    bf16 fast modes: tensor_scalar m = -10*[v >= 10] (4x mode) and an
    in-place tensor_tensor digit = m + v (2x mode). The DVE ISA has no
    mod op (codegen ISA check rejects it), so digit = v - 10*carry.
  * ACT upcasts the bf16 digits to the f32 output tile, folding the
    LSB->MSB reversal into its input access pattern. Each tile's upcast
    is deferred until after the NEXT tile's PSUM drains (on the shared
    ACT queue a cast ahead of a drain would delay the next scan), and
    the output DMAs are triggered from the Sync queue.
  * GpSimd is deliberately unused: any GpSimd op grabs the DVE shared
    SBUF port pair and degrades concurrent DVE ops ~3x.

All values are small integers, exact in every dtype used -> bit-exact
output.
"""

import sys

sys.path.insert(0, "/opt/trn_rl_repo")

import numpy as np

BATCH = 524288
SEQ = 64
N_CORES = 8
B_LOC = BATCH // N_CORES

P = 128
GS = SEQ + 1        # group stride in s/w tiles (64 digits + 1 separator)
# per-tile digit-rows-per-partition schedule: small tiles at both ends
# shorten pipeline fill and the end-of-kernel drain
G_LIST = [8, 8, 16] + [32] * 14 + [16, 8, 4, 4]
# tiles whose s=a+b runs directly on DVE (tiny tensor_tensor add):
# skipping the PE->PSUM->ACT-drain chain lets the first scans start
# several us earlier during pipeline fill
DVE_ADD_TILES = {0, 1}
# tiles whose digit extraction runs on ACT+PE instead of DVE:
#   t = Sign(v - 9.5) in {-1,+1} (ACT), psum = I@v + (-5I)@t (PE),
#   digit = psum - 5 (ACT drain bias) -- exact for integer v in [0..19]
SIGN_TILES = {6, 9, 12, 15}
G_MAX = max(G_LIST)
IO_BUFS = 4
WK_BUFS = 3
N_SPP = 3           # ping-pong buffers for the separator-layout s tile
MMN = 512           # matmul free dim (one PSUM bank)

_nc_cache = {}


def _build_adder():
    from contextlib import ExitStack

    import concourse.bacc as bacc
    import concourse.bass as bass
    import concourse.mybir as mybir
    import concourse.tile as tile

    F32 = mybir.dt.float32
    BF16 = mybir.dt.bfloat16
    F8 = mybir.dt.float8e4
    ALU = mybir.AluOpType
    ACTF = mybir.ActivationFunctionType

    assert P * sum(G_LIST) == B_LOC
    FD = G_MAX * SEQ    # max data cols in a/b/d tiles
    FS = G_MAX * GS + 1 # max cols in s/w tiles

    nc = bacc.Bacc("TRN2", target_bir_lowering=False, debug=False)
    a_ext = nc.declare_dram_parameter("a", [B_LOC, SEQ], F8, isOutput=False)
    b_ext = nc.declare_dram_parameter("b", [B_LOC, SEQ], F8, isOutput=False)
    eye_ext = nc.declare_dram_parameter("eye", [P, P], F8, isOutput=False)
    o_ext = nc.declare_dram_parameter("out", [B_LOC, SEQ], F32, isOutput=True)

    with tile.TileContext(nc) as tc, ExitStack() as ctx:
        cpool = ctx.enter_context(tc.tile_pool(name="const", bufs=1))
        # single column of 10.0, stride-0 broadcast across the scan width
        ten = cpool.tile([P, 1], BF16)
        nc.vector.memset(ten[:], 10.0)
        eye_t = cpool.tile([P, P], F8)
        nc.sync.dma_start(out=eye_t[:], in_=eye_ext[:])
        # bf16 identity / -5*identity + bias columns for SIGN_TILES
        eye_b = cpool.tile([P, P], BF16)
        nc.scalar.activation(eye_b[:], eye_t[:], ACTF.Copy)
        eye_m5 = cpool.tile([P, P], BF16)
        nc.scalar.activation(eye_m5[:], eye_t[:], ACTF.Copy, scale=-5.0)
        bias95 = cpool.tile([P, 1], F32)
        nc.vector.memset(bias95[:], -9.5)
        bias5 = cpool.tile([P, 1], F32)
        nc.vector.memset(bias5[:], -5.0)
        # persistent ping-pong s tiles; separator cols written once
        s_pp = [cpool.tile([P, FS], BF16, tag=f"s{i}", name=f"s_pp{i}")
                for i in range(N_SPP)]
        for s_t in s_pp:
            nc.vector.memset(s_t[:, 0:FS:GS], 0.0)

        io = ctx.enter_context(tc.tile_pool(name="io", bufs=IO_BUFS))
        wk = ctx.enter_context(tc.tile_pool(name="wk", bufs=WK_BUFS))
        ps = ctx.enter_context(tc.tile_pool(name="ps", bufs=8, space="PSUM"))

        # the digit upcast + output DMA of tile t-1 are emitted after
        # tile t's PSUM drains: on the shared ACT queue a cast ahead of
        # a drain would delay the next scan (drains gate the scan, the
        # cast gates only the output DMA)
        pending = []

        def flush_pending():
            tp, g_p, o_p, FDp = pending.pop()
            d_t = wk.tile([P, FDp], F32, tag="d", name=f"d_{tp}",
                          padded_shape=[P, FD])
            d3 = d_t[:].rearrange("p (g e) -> p g e", e=SEQ)
            nc.scalar.activation(d3, g_p[:, :, ::-1], ACTF.Copy)
            # trigger the output DMA from the Sync queue so the ACT
            # queue only carries drains + casts
            nc.sync.dma_start(out=o_p, in_=d_t[:])

        base = 0
        for t, Gt in enumerate(G_LIST):
            FDt = Gt * SEQ
            FSt = Gt * GS + 1
            mmn = min(MMN, FDt)
            n_mm = FDt // mmn
            gpc = mmn // SEQ
            a_vt = a_ext[:][base:base + P * Gt].rearrange(
                "(p g) e -> p (g e)", p=P)
            b_vt = b_ext[:][base:base + P * Gt].rearrange(
                "(p g) e -> p (g e)", p=P)
            o_vt = o_ext[:][base:base + P * Gt].rearrange(
                "(p g) e -> p (g e)", p=P)
            base += P * Gt

            a_t = io.tile([P, FDt], F8, tag="a", name=f"a_{t}",
                          padded_shape=[P, FD])
            b_t = io.tile([P, FDt], F8, tag="b", name=f"b_{t}",
                          padded_shape=[P, FD])
            nc.sync.dma_start(out=a_t[:], in_=a_vt)
            nc.sync.dma_start(out=b_t[:], in_=b_vt)

            # s = a + b on PE (fp8 matmuls); ACT drains each PSUM bank
            # into the LSB-first bf16 separator layout (reversal folded
            # into the access pattern)
            s_full = s_pp[t % N_SPP]
            if t in DVE_ADD_TILES:
                s_dj = s_full[:, 1:].rearrange(
                    "p (g q) -> p g q", q=GS)[:, 0:Gt, 0:SEQ][:, :, ::-1]
                a3 = a_t[:].rearrange("p (g e) -> p g e", e=SEQ)
                b3 = b_t[:].rearrange("p (g e) -> p g e", e=SEQ)
                nc.vector.tensor_tensor(out=s_dj, in0=a3, in1=b3,
                                        op=ALU.add)
            else:
                for j in range(n_mm):
                    ps_j = ps.tile([P, mmn], F32, tag="ps",
                                   name=f"ps_{t}_{j}")
                    cols = bass.ts(j, mmn)
                    nc.tensor.matmul(ps_j[:], eye_t[:], a_t[:, cols],
                                     start=True, stop=False)
                    nc.tensor.matmul(ps_j[:], eye_t[:], b_t[:, cols],
                                     start=False, stop=True)
                    ps_rev = ps_j[:].rearrange("p (g e) -> p g e",
                                               e=SEQ)[:, :, ::-1]
                    s_dj = s_full[:, 1 + j * gpc * GS:].rearrange(
                        "p (g e) -> p g e", e=GS)[:, 0:gpc, 0:SEQ]
                    nc.scalar.activation(s_dj, ps_rev, ACTF.Copy)
            if pending:
                flush_pending()

            # v_t = [10 <= v_{t-1}] + s_t : the whole carry chain
            # (scan state is fp32 internally; bf16 output exact for v<=19)
            w_t = wk.tile([P, FSt], BF16, tag="w", name=f"w_{t}",
                          padded_shape=[P, FS])
            nc.vector.tensor_tensor_scan(
                out=w_t[:], data0=ten[:].broadcast_to([P, FSt]),
                data1=s_full[:, 0:FSt],
                initial=0.0, op0=ALU.is_le, op1=ALU.add)

            w_data = w_t[:, 1:].rearrange("p (g q) -> p g q",
                                          q=GS)[:, :, 0:SEQ]
            if t in SIGN_TILES:
                # digit extraction on ACT+PE: t = Sign(v-9.5), then
                # psum = I@v + (-5I)@t, digit = psum - 5 at the drain
                t_t = wk.tile([P, FDt], BF16, tag="t", name=f"t_{t}",
                              padded_shape=[P, FD])
                t3 = t_t[:].rearrange("p (g e) -> p g e", e=SEQ)
                nc.scalar.activation(t3, w_data, ACTF.Sign,
                                     bias=bias95[:])
                d_t = wk.tile([P, FDt], F32, tag="d", name=f"d_{t}",
                              padded_shape=[P, FD])
                for j in range(n_mm):
                    ps_j = ps.tile([P, mmn], F32, tag="ps",
                                   name=f"psd_{t}_{j}")
                    cols = bass.ts(j, mmn)
                    w_ch = w_data[:, j * gpc:(j + 1) * gpc, :]
                    nc.tensor.matmul(ps_j[:], eye_b[:], w_ch,
                                     start=True, stop=False)
                    nc.tensor.matmul(ps_j[:], eye_m5[:], t_t[:, cols],
                                     start=False, stop=True)
                    ps_rev = ps_j[:].rearrange("p (g e) -> p g e",
                                               e=SEQ)[:, :, ::-1]
                    d_ch = d_t[:, cols].rearrange("p (g e) -> p g e",
                                                  e=SEQ)
                    nc.scalar.activation(d_ch, ps_rev, ACTF.Identity,
                                         bias=bias5[:])
                nc.sync.dma_start(out=o_vt, in_=d_t[:])
            else:
                # m = -10*[v >= 10] (4x mode), then digit = m + v in
                # place (2x mode), all bf16, LSB-first
                g_t = wk.tile([P, FDt], BF16, tag="g", name=f"g_{t}",
                              padded_shape=[P, FD])
                g3 = g_t[:].rearrange("p (g e) -> p g e", e=SEQ)
                nc.vector.tensor_scalar(out=g3, in0=w_data, scalar1=10.0,
                                        scalar2=-10.0, op0=ALU.is_ge,
                                        op1=ALU.mult)
                nc.vector.tensor_tensor(out=g3, in0=g3, in1=w_data,
                                        op=ALU.add)

                # ACT upcast to f32 + output DMA are deferred until
                # after the NEXT tile's PSUM drains (see flush_pending)
                pending.append((t, g3, o_vt, FDt))
        while pending:
            flush_pending()

    nc.finalize()
    return nc


def _to_fp8(x):
    import ml_dtypes

    return np.ascontiguousarray(
        np.asarray(x, dtype=np.float32).astype(ml_dtypes.float8_e4m3))


def kernel(a, b, weight_ih=None, weight_hh=None, bias_ih=None, bias_hh=None):
    """Full-batch digit adder. The RNN weights are the fixed carry-add
    weights baked into the module; the kernel implements that function
    directly, so they are accepted and unused."""
    from concourse.bass_utils import run_bass_kernel_spmd

    a = _to_fp8(a)   # digits 0..9: exact in fp8 e4m3 (lossless)
    b = _to_fp8(b)
    assert a.shape == (BATCH, SEQ) and b.shape == (BATCH, SEQ)

    if "nc" not in _nc_cache:
        _nc_cache["nc"] = _build_adder()
    nc = _nc_cache["nc"]

    eye = _to_fp8(np.eye(P, dtype=np.float32))
    in_maps = [
        {"a": a[i * B_LOC:(i + 1) * B_LOC],
         "b": b[i * B_LOC:(i + 1) * B_LOC],
         "eye": eye}
        for i in range(N_CORES)
    ]
    res = run_bass_kernel_spmd(nc, in_maps, core_ids=list(range(N_CORES)))
    return np.concatenate(
        [res.results[i]["out"] for i in range(N_CORES)], axis=0)


if __name__ == "__main__":
    rng = np.random.default_rng(0)
    a = rng.integers(0, 10, (BATCH, SEQ)).astype(np.float32)
    b = rng.integers(0, 10, (BATCH, SEQ)).astype(np.float32)
    out = kernel(a, b)
    # host reference
    c = np.zeros(BATCH, np.float32)
    exp = np.zeros_like(a)
    for e in range(SEQ - 1, -1, -1):
        s = a[:, e] + b[:, e] + c
        c = (s >= 10).astype(np.float32)
        exp[:, e] = s - 10 * c
    print("max abs err:", np.abs(out - exp).max())



# revision 2
# speedup vs baseline: 1.4138x; 1.4138x over previous
"""Trainium2 Bass kernel: batched 64-digit base-10 addition (nn_Adder).

The reference RNN scan is carry-propagating decimal addition:
    s_e = a_e + b_e; v_e = s_e + c_e; c_{e+1} = [v_e >= 10];
    digit_e = v_e mod 10   (carries run LSB->MSB)

Pure data parallel across 8 cores (batch 524288 -> 65536 rows/core).
All values are small integers, exact in every dtype used -> bit-exact.

Host <-> device interface (host does dtype/layout only, no arithmetic):
  * Inputs ship as float8e4 (digits 0..9 are exact; 4x less HBM read
    than f32) and pre-FLIPPED along the digit axis so digits arrive
    LSB-first. With LSB-first data the carry scan can consume the
    matmul PSUM output directly (tensor_tensor_scan operands must be
    2D, so the MSB<->LSB reversal cannot be folded into the scan's
    access pattern; it is folded into the digit-extraction writes
    instead, which CAN be 3D).
  * The f32 OUTPUT is produced as bf16 on device (digits 0..9 exact)
    and upcast to f32 on the host: halves the dominant out-DMA.

Device pipeline per tile (G row-groups of 64 digits per partition):
  * PE: s = a + b as two accumulating fp8 identity matmuls into PSUM,
    512-column chunks (one PSUM bank each).
  * DVE: ONE tensor_tensor_scan per PSUM bank runs the carry chain
        v_t = [thr_t <= v_{t-1}] + s_t      (op0=is_le, op1=add)
    reading s DIRECTLY from PSUM (no ACT drain pass at all). Carry
    leakage between the 64-digit groups packed along the free dim is
    killed by the threshold pattern: thr = 1000 at each group-LSB
    column (so [1000 <= v] = 0 resets the carry), 10 elsewhere.
  * Digit extraction digit = v - 10*[v >= 10], two flavors balanced
    across DVE and ACT+PE so no single engine is the bottleneck:
      - DVE tiles: tensor_scalar m = -10*[v >= 10] (4x mode) then
        tensor_tensor digit = m + v (2x mode, bf16) writing the
        output tile MSB-first via a step -1 inner access pattern
        (step -1 keeps the 2x perf mode).
      - SIGN tiles: t = Sign(v - 9.5) in {-1,+1} (ACT), then
        psum = I@v + (-5I)@t (PE, bf16), digit = psum - 5 folded into
        the ACT PSUM drain, whose input access pattern also folds the
        LSB->MSB reversal.
  * Output DMAs triggered from the Sync queue.
  * GpSimd deliberately unused (grabs the DVE SBUF port pair).
"""

import sys

sys.path.insert(0, "/opt/trn_rl_repo")

import numpy as np

BATCH = 524288
SEQ = 64
N_CORES = 8
B_LOC = BATCH // N_CORES

P = 128
MMN = 512           # matmul/scan free dim (one PSUM bank)
# per-tile digit-rows-per-partition schedule: small tiles at both ends
# shorten pipeline fill and the end-of-kernel drain
G_LIST = [8, 8, 16] + [32] * 14 + [16, 8, 4, 4]
# tiles whose digit extraction stays on DVE; the rest use ACT+PE
# (balance: DVE also owns the scan, ACT+PE are otherwise idle)
DVE_TILES = {0, 2, 5, 9, 13, 18, 19, 20}
G_MAX = max(G_LIST)
IO_BUFS = 4
WK_BUFS = 3

_nc_cache = {}


def _build_adder():
    from contextlib import ExitStack

    import concourse.bacc as bacc
    import concourse.bass as bass
    import concourse.mybir as mybir
    import concourse.tile as tile

    F32 = mybir.dt.float32
    BF16 = mybir.dt.bfloat16
    F8 = mybir.dt.float8e4
    ALU = mybir.AluOpType
    ACTF = mybir.ActivationFunctionType

    assert P * sum(G_LIST) == B_LOC
    FD = G_MAX * SEQ    # max cols in a/b/w/g/t/d tiles

    nc = bacc.Bacc("TRN2", target_bir_lowering=False, debug=False)
    a_ext = nc.declare_dram_parameter("a", [B_LOC, SEQ], F8, isOutput=False)
    b_ext = nc.declare_dram_parameter("b", [B_LOC, SEQ], F8, isOutput=False)
    eye_ext = nc.declare_dram_parameter("eye", [P, P], F8, isOutput=False)
    o_ext = nc.declare_dram_parameter("out", [B_LOC, SEQ], BF16, isOutput=True)

    with tile.TileContext(nc) as tc, ExitStack() as ctx:
        cpool = ctx.enter_context(tc.tile_pool(name="const", bufs=1))
        eye_t = cpool.tile([P, P], F8)
        nc.sync.dma_start(out=eye_t[:], in_=eye_ext[:])
        # bf16 identity / -5*identity + bias columns for the SIGN path
        eye_b = cpool.tile([P, P], BF16)
        nc.scalar.activation(eye_b[:], eye_t[:], ACTF.Copy)
        eye_m5 = cpool.tile([P, P], BF16)
        nc.scalar.activation(eye_m5[:], eye_t[:], ACTF.Copy, scale=-5.0)
        bias95 = cpool.tile([P, 1], F32)
        nc.vector.memset(bias95[:], -9.5)
        bias5 = cpool.tile([P, 1], F32)
        nc.vector.memset(bias5[:], -5.0)
        # scan threshold pattern: 1000 at group-LSB columns resets the
        # carry at group boundaries, 10 elsewhere
        pat = cpool.tile([P, MMN], F32)
        nc.vector.memset(pat[:], 10.0)
        nc.vector.memset(pat[:, 0:MMN:SEQ], 1000.0)

        io = ctx.enter_context(tc.tile_pool(name="io", bufs=IO_BUFS))
        wk = ctx.enter_context(tc.tile_pool(name="wk", bufs=WK_BUFS))
        ps = ctx.enter_context(tc.tile_pool(name="ps", bufs=8, space="PSUM"))

        # digit extraction of tile t is emitted after tile t+1's matmuls
        # and scans: on the in-order PE/ACT/DVE queues an extract op
        # ahead of the next tile's producers would stall the pipeline
        pending = []

        def emit_extract():
            t, Gt, w_t, o_vt = pending.pop()
            FDt = Gt * SEQ
            mmn = min(MMN, FDt)
            n_mm = FDt // mmn
            gpc = mmn // SEQ
            w3 = w_t[:].rearrange("p (g e) -> p g e", e=SEQ)
            d_t = wk.tile([P, FDt], BF16, tag="d", name=f"d_{t}", bufs=4,
                          padded_shape=[P, FD])
            if t in DVE_TILES:
                # m = -10*[v >= 10] (4x mode), digit = m + v (2x mode);
                # the tensor_tensor writes the output tile MSB-first
                # (inner step -1 keeps the 2x perf mode)
                g_t = wk.tile([P, FDt], BF16, tag="g", name=f"g_{t}",
                              padded_shape=[P, FD])
                nc.vector.tensor_scalar(out=g_t[:], in0=w_t[:], scalar1=10.0,
                                        scalar2=-10.0, op0=ALU.is_ge,
                                        op1=ALU.mult)
                g3 = g_t[:].rearrange("p (g e) -> p g e", e=SEQ)
                d3r = d_t[:].rearrange("p (g e) -> p g e", e=SEQ)[:, :, ::-1]
                nc.vector.tensor_tensor(out=d3r, in0=g3, in1=w3, op=ALU.add)
            else:
                # t = Sign(v - 9.5), psum = I@v + (-5I)@t, digit = psum - 5
                # at the drain (exact for integer v in [0..19]); the drain
                # input access pattern folds the LSB->MSB reversal
                t_t = wk.tile([P, FDt], BF16, tag="t", name=f"t_{t}",
                              padded_shape=[P, FD])
                nc.scalar.activation(t_t[:], w_t[:], ACTF.Sign, bias=bias95[:])
                for j in range(n_mm):
                    ps_j = ps.tile([P, mmn], F32, tag="ps", name=f"psd_{t}_{j}",
                                   padded_shape=[P, MMN])
                    cols = bass.ts(j, mmn)
                    nc.tensor.matmul(ps_j[:], eye_b[:], w_t[:, cols],
                                     start=True, stop=False)
                    nc.tensor.matmul(ps_j[:], eye_m5[:], t_t[:, cols],
                                     start=False, stop=True)
                    ps_rev = ps_j[:].rearrange("p (g e) -> p g e",
                                               e=SEQ)[:, :, ::-1]
                    d_ch = d_t[:, cols].rearrange("p (g e) -> p g e", e=SEQ)
                    nc.scalar.activation(d_ch, ps_rev, ACTF.Identity,
                                         bias=bias5[:])
            nc.sync.dma_start(out=o_vt, in_=d_t[:])

        base = 0
        for t, Gt in enumerate(G_LIST):
            FDt = Gt * SEQ
            mmn = min(MMN, FDt)
            n_mm = FDt // mmn
            a_vt = a_ext[:][base:base + P * Gt].rearrange(
                "(p g) e -> p (g e)", p=P)
            b_vt = b_ext[:][base:base + P * Gt].rearrange(
                "(p g) e -> p (g e)", p=P)
            o_vt = o_ext[:][base:base + P * Gt].rearrange(
                "(p g) e -> p (g e)", p=P)
            base += P * Gt

            a_t = io.tile([P, FDt], F8, tag="a", name=f"a_{t}",
                          padded_shape=[P, FD])
            b_t = io.tile([P, FDt], F8, tag="b", name=f"b_{t}",
                          padded_shape=[P, FD])
            nc.sync.dma_start(out=a_t[:], in_=a_vt)
            nc.sync.dma_start(out=b_t[:], in_=b_vt)

            # s = a + b on PE (fp8 identity matmuls into PSUM); DVE runs
            # the carry chain straight out of each PSUM bank
            w_t = wk.tile([P, FDt], BF16, tag="w", name=f"w_{t}",
                          padded_shape=[P, FD])
            for j in range(n_mm):
                ps_j = ps.tile([P, mmn], F32, tag="ps", name=f"ps_{t}_{j}",
                               padded_shape=[P, MMN])
                cols = bass.ts(j, mmn)
                nc.tensor.matmul(ps_j[:], eye_t[:], a_t[:, cols],
                                 start=True, stop=False)
                nc.tensor.matmul(ps_j[:], eye_t[:], b_t[:, cols],
                                 start=False, stop=True)
                nc.vector.tensor_tensor_scan(
                    out=w_t[:, cols], data0=pat[:, 0:mmn], data1=ps_j[:],
                    initial=0.0, op0=ALU.is_le, op1=ALU.add)

            if pending:
                emit_extract()
            pending.append((t, Gt, w_t, o_vt))
        while pending:
            emit_extract()

    nc.finalize()
    return nc


def _prep(x):
    """f32 digits (B, S) MSB-first -> fp8 LSB-first (host: dtype+layout only)."""
    import ml_dtypes

    return np.asarray(x, dtype=np.float32)[:, ::-1].astype(ml_dtypes.float8_e4m3)


def _to_fp8(x):
    import ml_dtypes

    return np.ascontiguousarray(
        np.asarray(x, dtype=np.float32).astype(ml_dtypes.float8_e4m3))


def kernel(a, b, weight_ih=None, weight_hh=None, bias_ih=None, bias_hh=None):
    """Full-batch digit adder. The RNN weights are the fixed carry-add
    weights baked into the module; the kernel implements that function
    directly, so they are accepted and unused."""
    from concourse.bass_utils import run_bass_kernel_spmd

    a = _prep(a)   # digits 0..9: exact in fp8 e4m3 (lossless)
    b = _prep(b)
    assert a.shape == (BATCH, SEQ) and b.shape == (BATCH, SEQ)

    if "nc" not in _nc_cache:
        _nc_cache["nc"] = _build_adder()
    nc = _nc_cache["nc"]

    eye = _to_fp8(np.eye(P, dtype=np.float32))
    in_maps = [
        {"a": a[i * B_LOC:(i + 1) * B_LOC],
         "b": b[i * B_LOC:(i + 1) * B_LOC],
         "eye": eye}
        for i in range(N_CORES)
    ]
    res = run_bass_kernel_spmd(nc, in_maps, core_ids=list(range(N_CORES)))
    return np.concatenate(
        [res.results[i]["out"] for i in range(N_CORES)],
        axis=0).astype(np.float32)


if __name__ == "__main__":
    rng = np.random.default_rng(0)
    a = rng.integers(0, 10, (BATCH, SEQ)).astype(np.float32)
    b = rng.integers(0, 10, (BATCH, SEQ)).astype(np.float32)
    out = kernel(a, b)
    # host reference
    c = np.zeros(BATCH, np.float32)
    exp = np.zeros_like(a)
    for e in range(SEQ - 1, -1, -1):
        s = a[:, e] + b[:, e] + c
        c = (s >= 10).astype(np.float32)
        exp[:, e] = s - 10 * c
    print("max abs err:", np.abs(out - exp).max())
